# revision 1
# baseline (speedup 1.0000x reference)
"""Trainium2 Bass kernel for nn_Attention_13829794693777.

Multi-head attention (8 heads, head_dim 48) + LePE depthwise 3x3 conv on v.
Sharding: tensor-parallel over heads -- one head per NeuronCore (8 cores).
Each core gets the full (transposed, bf16) input activations plus its head's
qkv weight slice and LePE channel slice; it produces the [seq, 48] channel
slice of the output. The host concatenates slices and reshapes.

Device-side dataflow (per core / head):
  1. Projection: qT/kT channel-major [128(dup), seq] bf16 via W-stationary
     matmuls (q and k duplicated at partitions 0-47 and 64-111 so scores
     matmuls can row-group-pack), v channel-major into a 66x66 zero-padded
     image buffer (vT_pad) for the conv.
  2. v_aug [seq-tile, 49] bf16 (seq-major v + ones column) via PE transposes.
  3. Scores transposed: S^T[k, q] = kT.T-slice @ qT (K=48), exp on ACT
     (no max subtraction; |S| <= ~15 so exp is fp32-safe), giving P^T bf16.
  4. PV: out^T[ch, q] += v_aug[kt].T @ P^T[kt] with 2x column tiling
     (even k-tiles -> psum rows 0-48, odd -> 64-112). Ones column of v_aug
     yields the softmax denominators as row 48.
  5. LePE: 9 shifted multiply-accumulate taps on DVE over the padded image.
  6. Epilogue per 512-query chunk: sum the two psum half-results, PE-transpose
     to seq-major, divide by denominator, add (transposed) LePE, DMA out.
"""

import numpy as np
import ml_dtypes

NUM_HEADS = 8
DIM = 384
HD = 48
B = 2
N = 4096
SEQ = B * N          # 8192
IMG = 64             # H = W = 64
PADW = IMG + 2       # 66
PADN = PADW * PADW   # 4356
SCALE = HD ** -0.5
NCHUNK = SEQ // 512  # 16 query chunks of 512
KT_PER_B = N // 128  # 32 k-tiles per batch

_CACHE = {}


def _build_module():
    """Build (once) the Bacc module shared by all 8 cores."""
    import concourse.bacc as bacc
    import concourse.bass as bass
    import concourse.mybir as mybir
    import concourse.tile as tile
    from concourse.dve_ops import AFFINE_THEN_ADD
    from contextlib import ExitStack

    dt = mybir.dt
    AF = mybir.ActivationFunctionType
    ALU = mybir.AluOpType

    nc = bacc.Bacc("TRN2", target_bir_lowering=False, debug=False, num_devices=8)

    # ---- DRAM parameters -------------------------------------------------
    xT_d = nc.dram_tensor("xT", [3, 128, SEQ], dt.bfloat16, kind="ExternalInput").ap()
    wqd_d = nc.dram_tensor("wqd", [3, 128, 128], dt.bfloat16, kind="ExternalInput").ap()
    wkd_d = nc.dram_tensor("wkd", [3, 128, 128], dt.bfloat16, kind="ExternalInput").ap()
    wv_d = nc.dram_tensor("wv", [3, 128, HD], dt.bfloat16, kind="ExternalInput").ap()
    bqd_d = nc.dram_tensor("bqd", [128, 1], dt.float32, kind="ExternalInput").ap()
    bkd_d = nc.dram_tensor("bkd", [128, 1], dt.float32, kind="ExternalInput").ap()
    bvd_d = nc.dram_tensor("bvd", [128, 1], dt.float32, kind="ExternalInput").ap()
    lw9_d = nc.dram_tensor("lw9", [128, 9], dt.float32, kind="ExternalInput").ap()
    lb1_d = nc.dram_tensor("lb1", [128, 1], dt.float32, kind="ExternalInput").ap()
    idn_d = nc.dram_tensor("idn", [128, 64], dt.float32, kind="ExternalInput").ap()
    out_d = nc.dram_tensor("out", [64, 128, HD], dt.float32, kind="ExternalOutput").ap()
    # seq-tile-major output view iterated (partition, tile, ch)
    out_v = out_d.rearrange("t p c -> p t c")

    with tile.TileContext(nc) as tc, ExitStack() as ctx:
        per = ctx.enter_context(tc.tile_pool(name="per", bufs=1))
        rot = ctx.enter_context(tc.tile_pool(name="rot", bufs=2))
        ptp = ctx.enter_context(tc.tile_pool(name="ptp", bufs=6))

        # ---- persistent SBUF tensors ------------------------------------
        xs = [[per.tile([128, 2048], dt.bfloat16, name=f"x{c}_{j}", tag=f"x{c}_{j}") for j in range(4)]
              for c in range(3)]
        qTd = [per.tile([128, 512], dt.bfloat16, name=f"q{n}", tag=f"q{n}") for n in range(NCHUNK)]
        kTd = [per.tile([128, 512], dt.bfloat16, name=f"k{n}", tag=f"k{n}") for n in range(NCHUNK)]
        v_aug = per.tile([128, 64 * 49], dt.bfloat16, name="vaug", tag="vaug")
        v_aug3 = v_aug[:].rearrange("p (t c) -> p t c", c=49)
        vT_pad = per.tile([128, PADN], dt.float32, name="vpad", tag="vpad")
        vp3 = vT_pad[:].rearrange("p (a b) -> p a b", b=PADW)
        vT_dense = per.tile([128, N], dt.bfloat16, name="vdense", tag="vdense")
        acc0 = per.tile([128, N], dt.float32, name="acc0", tag="acc0")
        acc1 = per.tile([128, N], dt.float32, name="acc1", tag="acc1")

        wq_sb = [per.tile([128, 128], dt.bfloat16, name=f"wq{c}", tag=f"wq{c}") for c in range(3)]
        wk_sb = [per.tile([128, 128], dt.bfloat16, name=f"wk{c}", tag=f"wk{c}") for c in range(3)]
        wv_sb = [per.tile([128, HD], dt.bfloat16, name=f"wv{c}", tag=f"wv{c}") for c in range(3)]
        bq_sb = per.tile([128, 1], dt.float32, name="bq", tag="bq")
        bk_sb = per.tile([128, 1], dt.float32, name="bk", tag="bk")
        bv_sb = per.tile([128, 1], dt.float32, name="bv", tag="bv")
        lw_sb = per.tile([128, 9], dt.float32, name="lw", tag="lw")
        lb_sb = per.tile([128, 1], dt.float32, name="lb", tag="lb")
        id_sb = per.tile([128, 64], dt.float32, name="idn", tag="idn")
        idb_sb = per.tile([128, 64], dt.bfloat16, name="idnb", tag="idnb")

        # ---- input DMAs --------------------------------------------------
        for j in range(4):
            for c in range(3):
                nc.sync.dma_start(xs[c][j][:], xT_d[c, :, j * 2048:(j + 1) * 2048])
        for c in range(3):
            nc.sync.dma_start(wq_sb[c][:], wqd_d[c])
            nc.sync.dma_start(wk_sb[c][:], wkd_d[c])
            nc.sync.dma_start(wv_sb[c][:], wv_d[c])
        nc.sync.dma_start(bq_sb[:], bqd_d[:])
        nc.sync.dma_start(bk_sb[:], bkd_d[:])
        nc.sync.dma_start(bv_sb[:], bvd_d[:])
        nc.sync.dma_start(lw_sb[:], lw9_d[:])
        nc.sync.dma_start(lb_sb[:], lb1_d[:])
        nc.sync.dma_start(id_sb[:], idn_d[:])
        nc.vector.tensor_copy(idb_sb[:], id_sb[:])

        # zero the padded image (borders must be 0)
        nc.vector.memset(vT_pad[:], 0.0)
        nc.vector.memset(v_aug3[:, :, 48:49], 1.0)

        # ---- phase 1: projection ----------------------------------------
        psA_ctx = ExitStack()
        psA = psA_ctx.enter_context(tc.tile_pool(name="psA", bufs=2, space="PSUM"))
        for n in range(NCHUNK):
            rhs = [xs[c][n // 4][:, (n % 4) * 512:(n % 4 + 1) * 512] for c in range(3)]
            pq = psA.tile([128, 512], dt.float32, name="pq", tag="pq")
            for c in range(3):
                nc.tensor.matmul(pq[:], wq_sb[c][:], rhs[c],
                                 start=(c == 0), stop=(c == 2))
            nc.vector.tensor_scalar(qTd[n][:], pq[:], bq_sb[:, 0:1], None, op0=ALU.add)
            pk = psA.tile([128, 512], dt.float32, name="pk", tag="pk")
            for c in range(3):
                nc.tensor.matmul(pk[:], wk_sb[c][:], rhs[c],
                                 start=(c == 0), stop=(c == 2))
            nc.vector.tensor_scalar(kTd[n][:], pk[:], bk_sb[:, 0:1], None, op0=ALU.add)

            b = n // 8
            rb = 64 * b
            pvv = psA.tile([128, 512], dt.float32, name="pvv", tag="pvv")
            for c in range(3):
                nc.tensor.matmul(pvv[rb:rb + HD, :], wv_sb[c][:], rhs[c],
                                 start=(c == 0), stop=(c == 2),
                                 tile_position=(0, rb))
            # scatter the 512 pixels (8 image rows) into the padded image
            r0 = 8 * (n % 8)
            dest = vp3[rb:rb + HD, 1 + r0:1 + r0 + 8, 1:65]
            nc.vector.tensor_scalar(dest, pvv[rb:rb + HD, :], bv_sb[rb:rb + HD, 0:1],
                                    None, op0=ALU.add)
            # dense channel-major copy (transpose source for v_aug)
            p0 = 512 * (n % 8)
            nc.vector.tensor_scalar(vT_dense[rb:rb + HD, p0:p0 + 512],
                                    pvv[rb:rb + HD, :], bv_sb[rb:rb + HD, 0:1],
                                    None, op0=ALU.add)

        # ---- phase 2: v_aug via PE transposes ---------------------------
        for t in range(64):
            b = t // 32
            rb = 64 * b
            tt = t % 32
            tq = psA.tile([128, HD], dt.bfloat16, name="tq", tag="tq")
            nc.tensor.matmul(tq[:], vT_dense[rb:rb + HD, 128 * tt:128 * tt + 128],
                             idb_sb[rb:rb + HD, 0:HD],
                             is_transpose=True, tile_position=(rb, 0))
            nc.vector.tensor_copy(v_aug3[:, t, 0:HD], tq[:])
        psA_ctx.close()

        # ---- phase 3: LePE taps on DVE ----------------------------------
        from concourse.bass import AP  # noqa: F401
        taps = [(dr, dc) for dr in (-1, 0, 1) for dc in (-1, 0, 1)]
        ctr = taps.index((0, 0))

        def tap_ap(dr, dc):
            return vp3[0:112, 1 + dr:1 + dr + IMG, 1 + dc:1 + dc + IMG]

        def tap_q(dr, dc, q):
            return vp3[0:112, 1 + 16 * q + dr:1 + 16 * q + 16 + dr,
                       1 + dc:1 + dc + IMG]

        for q in range(4):
            fr = slice(1024 * q, 1024 * q + 1024)
            nc.vector.tensor_scalar(acc0[0:112, fr], tap_q(0, 0, q),
                                    lw_sb[0:112, ctr:ctr + 1], lb_sb[0:112, 0:1],
                                    op0=ALU.mult, op1=ALU.add)
            cur, oth = acc0, acc1
            for i, (dr, dc) in enumerate(taps):
                if (dr, dc) == (0, 0):
                    continue
                nc.vector._custom_dve(AFFINE_THEN_ADD, out=oth[0:112, fr],
                                      in0=tap_q(dr, dc, q), in1=cur[0:112, fr],
                                      s0=lw_sb[0:112, i:i + 1], s1=0.0)
                cur, oth = oth, cur
        lepe = cur  # == acc0 after 8 swaps per quarter

        # ---- phase 4: main attention loop -------------------------------
        psB = ctx.enter_context(tc.tile_pool(name="psB", bufs=2, space="PSUM"))
        groups = [list(range(g, min(g + 3, KT_PER_B))) for g in range(0, KT_PER_B, 3)]
        for cc in range(NCHUNK):
            bc = cc // 8
            pv = psB.tile([128, 512], dt.float32, name="pv", tag="pv")
            for kts in groups:
                st = psB.tile([128, 1536], dt.float32, name="st", tag="st")
                for j, kt in enumerate(kts):
                    row = 64 * (kt & 1)
                    ktile = kTd[bc * 8 + kt // 4]
                    koff = (kt % 4) * 128
                    nc.tensor.matmul(st[:, j * 512:(j + 1) * 512],
                                     ktile[row:row + HD, koff:koff + 128],
                                     qTd[cc][row:row + HD, :],
                                     tile_position=(row, 0))
                w = 512 * len(kts)
                pt = ptp.tile([128, 1536], dt.bfloat16, name="pt", tag="pt")
                nc.scalar.activation(pt[:, 0:w], st[:, 0:w], AF.Exp)
                for j, kt in enumerate(kts):
                    colb = 64 * (kt & 1)
                    nc.tensor.matmul(pv[colb:colb + 49, :],
                                     v_aug3[:, bc * 32 + kt, 0:49],
                                     pt[:, j * 512:(j + 1) * 512],
                                     start=(kt <= 1), stop=(kt >= KT_PER_B - 2),
                                     tile_position=(0, colb),
                                     skip_group_check=True)

            # ---- epilogue for this 512-query chunk ----------------------
            # stage layout: attn-even @0..195, attn-odd @196..391 (bank 0),
            # lepe @512..703 (bank 1)
            halves = rot.tile([128, 512], dt.float32, name="halves", tag="halves")
            nc.vector.tensor_copy(halves[0:49, :], pv[0:49, :])
            nc.vector.tensor_copy(halves[64:113, :], pv[64:113, :])
            stage = psB.tile([128, 1536], dt.float32, name="st", tag="st")
            for qs in range(4):
                nc.tensor.matmul(stage[:, qs * 49:qs * 49 + 49],
                                 halves[0:49, qs * 128:(qs + 1) * 128],
                                 id_sb[0:49, 0:49],
                                 is_transpose=True, tile_position=(0, 0))
                nc.tensor.matmul(stage[:, 512 + qs * 49:512 + qs * 49 + 49],
                                 halves[64:113, qs * 128:(qs + 1) * 128],
                                 id_sb[64:113, 0:49],
                                 is_transpose=True, tile_position=(64, 0))
                tglob = 4 * cc + qs
                rb = 64 * bc
                tt = tglob % 32
                nc.tensor.matmul(stage[:, 1024 + qs * 48:1024 + qs * 48 + 48],
                                 lepe[rb:rb + HD, 128 * tt:128 * tt + 128],
                                 id_sb[rb:rb + HD, 0:HD],
                                 is_transpose=True, tile_position=(rb, 0))
            tmp = rot.tile([128, 196], dt.float32, name="tmp", tag="tmp")
            rec = rot.tile([128, 4], dt.float32, name="rec", tag="rec")
            ot = rot.tile([128, 192], dt.float32, name="ot", tag="ot")
            nc.vector.tensor_copy(tmp[:], stage[:, 0:196])
            nc.vector.tensor_add(tmp[:], tmp[:], stage[:, 512:708])
            tmp3 = tmp[:].rearrange("p (a b) -> p a b", b=49)
            nc.vector.reciprocal(rec[:], tmp3[:, :, 48:49])
            for qs in range(4):
                nc.vector._custom_dve(AFFINE_THEN_ADD,
                                      out=ot[:, qs * 48:(qs + 1) * 48],
                                      in0=tmp[:, qs * 49:qs * 49 + 48],
                                      in1=stage[:, 1024 + qs * 48:1024 + qs * 48 + 48],
                                      s0=rec[:, qs:qs + 1], s1=0.0)
            nc.sync.dma_start(out_v[:, 4 * cc:4 * cc + 4, :],
                              ot[:].rearrange("p (t c) -> p t c", c=HD))

    nc.compile()
    return nc


def _prep_in_maps(x, qkv_w, qkv_b, lepe_w, lepe_b):
    bf16 = ml_dtypes.bfloat16
    X = np.asarray(x, dtype=np.float32).reshape(SEQ, DIM)
    xT = np.ascontiguousarray(X.T).astype(bf16).reshape(3, 128, SEQ)

    qkv_w = np.asarray(qkv_w, dtype=np.float32)
    qkv_b = np.asarray(qkv_b, dtype=np.float32)
    lepe_w = np.asarray(lepe_w, dtype=np.float32)
    lepe_b = np.asarray(lepe_b, dtype=np.float32)

    idn = np.zeros((128, 64), dtype=np.float32)
    idn[0:64, 0:64] = np.eye(64, dtype=np.float32)
    idn[64:128, 0:64] = np.eye(64, dtype=np.float32)

    in_maps = []
    for h in range(NUM_HEADS):
        sl = slice(h * HD, (h + 1) * HD)
        wq = qkv_w[sl, :] * SCALE                    # [48, 384]
        wk = qkv_w[DIM + h * HD:DIM + (h + 1) * HD, :]
        wv = qkv_w[2 * DIM + h * HD:2 * DIM + (h + 1) * HD, :]
        wqd = np.zeros((3, 128, 128), dtype=np.float32)
        wkd = np.zeros((3, 128, 128), dtype=np.float32)
        for c in range(3):
            wqd[c, :, 0:HD] = wq.T[c * 128:(c + 1) * 128]
            wqd[c, :, 64:64 + HD] = wq.T[c * 128:(c + 1) * 128]
            wkd[c, :, 0:HD] = wk.T[c * 128:(c + 1) * 128]
            wkd[c, :, 64:64 + HD] = wk.T[c * 128:(c + 1) * 128]
        wvc = np.ascontiguousarray(wv.T).reshape(3, 128, HD)

        def dupvec(v):
            o = np.zeros((128, 1), dtype=np.float32)
            o[0:HD, 0] = v
            o[64:64 + HD, 0] = v
            return o

        bq = dupvec(qkv_b[sl] * SCALE)
        bk = dupvec(qkv_b[DIM + h * HD:DIM + (h + 1) * HD])
        bv = dupvec(qkv_b[2 * DIM + h * HD:2 * DIM + (h + 1) * HD])
        lw = lepe_w[sl, 0].reshape(HD, 9)            # [48, 9] taps row-major
        lw9 = np.zeros((128, 9), dtype=np.float32)
        lw9[0:HD] = lw
        lw9[64:64 + HD] = lw
        lb = dupvec(lepe_b[sl])

        in_maps.append({
            "xT": xT,
            "wqd": wqd.astype(bf16),
            "wkd": wkd.astype(bf16),
            "wv": wvc.astype(bf16),
            "bqd": bq, "bkd": bk, "bvd": bv,
            "lw9": lw9, "lb1": lb, "idn": idn,
        })
    return in_maps


def kernel(x, qkv_w, qkv_b, lepe_w, lepe_b, H=64, W=64):
    assert int(H) == 64 and int(W) == 64
    from concourse.bass_utils import run_bass_kernel_spmd

    if "nc" not in _CACHE:
        _CACHE["nc"] = _build_module()
    nc = _CACHE["nc"]

    in_maps = _prep_in_maps(x, qkv_w, qkv_b, lepe_w, lepe_b)
    res = run_bass_kernel_spmd(nc, in_maps, core_ids=list(range(NUM_HEADS)))

    full = np.empty((SEQ, DIM), dtype=np.float32)
    for h in range(NUM_HEADS):
        full[:, h * HD:(h + 1) * HD] = res.results[h]["out"].reshape(SEQ, HD)
    return full.reshape(B, N, DIM)



# revision 32
# speedup vs baseline: 1.4504x; 1.4504x over previous
"""Trainium2 Bass kernel for nn_Attention_13829794693777.

Multi-head attention (8 heads, head_dim 48) + LePE depthwise 3x3 conv on v.
Sharding: tensor-parallel over heads -- one head per NeuronCore (8 cores).
Each core gets the full (transposed, bf16) input activations plus its head's
qkv weight slice and LePE channel slice; it produces the [seq, 48] channel
slice of the output. The host concatenates slices and reshapes.

Device-side dataflow (per core / head), designed to keep the Activation
engine (the exp() throughput floor, ~250us modeled) busy from ~6us onward:

  1. Projections are W-stationary matmuls into a single shared PSUM bank
     (psX), with biases folded into extra rank-1 matmuls (ones-vector
     tricks). The k bias is dropped entirely (softmax-invariant).
     q/k land channel-major [48, 512] per 512-pixel chunk -> SBUF bf16.
     v lands twice: seq-major v_aug tiles [128 pix, 49] (last col = 1.0
     via the bias matmul) for PV, and channel-major [48, 512] scattered
     into a zero-padded 66x66 image (vpad, bf16) for LePE.
  2. Scores transposed: st[k,q] = k-block(48x128).T @ qT(48x512), fp32
     PSUM, 3 k-tiles per group (two rotating 3-bank st tiles). exp on ACT
     (no max subtraction; |S| <= ~15) -> P^T bf16 in SBUF.
  3. PV P-stationary: out[q,ch] += ptBlock(128k x 128q).T @ v_aug(128k,49).
     Cost is 49 columns per matmul; the ones column yields softmax
     denominators at col 48. All 32 k-tiles accumulate into one PSUM bank
     (4 q-blocks x 49), already seq-major -- no output transposes.
  4. LePE: 9 diagonal-weight matmuls per q-block (diag(w_tap) as moving
     rhs, shifted padded-image view as stationary lhsT) + bias matmul,
     accumulated into region B of the same PSUM bank. Zero DVE tap work.
  5. Epilogue per 512-query chunk: reciprocal of denominators (DVE), then
     4x AFFINE_THEN_ADD: out = attn*rec + lepe -> SBUF -> DMA out.

Emission order interleaves batch-1 projections (and late batch-0 units)
between chunk score groups so the PE never starves ACT.
"""

import numpy as np
import ml_dtypes

NUM_HEADS = 8
DIM = 384
HD = 48
B = 2
N = 4096
SEQ = B * N          # 8192
IMG = 64             # H = W = 64
PADW = IMG + 2       # 66
PADN = PADW * PADW   # 4356
SCALE = HD ** -0.5
NCHUNK = SEQ // 512  # 16 query chunks of 512
KT_PER_B = N // 128  # 32 k-tiles per batch

_CACHE = {}


def _build_module():
    """Build (once) the Bacc module shared by all 8 cores."""
    import concourse.bacc as bacc
    import concourse.mybir as mybir
    import concourse.tile as tile
    from concourse.dve_ops import AFFINE_THEN_ADD
    from contextlib import ExitStack

    dt = mybir.dt
    AF = mybir.ActivationFunctionType
    ALU = mybir.AluOpType

    nc = bacc.Bacc("TRN2", target_bir_lowering=False, debug=False, num_devices=8)

    # ---- DRAM parameters -------------------------------------------------
    xT_d = nc.dram_tensor("xT", [3, 128, SEQ], dt.bfloat16, kind="ExternalInput").ap()
    # wall packs every weight: wq(3x48)|wk(3x48)|wv(3x49)|dg(9x48)|aux(785,row0)
    # + two per-partition bias columns (bq, bv) folded into the PSUM copies
    WN = 144 + 144 + 147 + 432 + 785
    wall_d = nc.dram_tensor("wall", [128, WN], dt.bfloat16, kind="ExternalInput").ap()
    bcol_d = nc.dram_tensor("bcol", [128, 2], dt.float32, kind="ExternalInput").ap()
    vone_d = nc.dram_tensor("vone", [1, PADN], dt.bfloat16, kind="ExternalInput").ap()
    out_d = nc.dram_tensor("out", [64, 128, HD], dt.float32, kind="ExternalOutput").ap()
    out_v = out_d.rearrange("t p c -> p t c")

    with tile.TileContext(nc) as tc, ExitStack() as ctx:
        per = ctx.enter_context(tc.tile_pool(name="per", bufs=1))
        rot = ctx.enter_context(tc.tile_pool(name="rot", bufs=2))
        ptp = ctx.enter_context(tc.tile_pool(name="ptp", bufs=3))
        pss = ctx.enter_context(tc.tile_pool(name="pss", bufs=2, space="PSUM"))
        psv = ctx.enter_context(tc.tile_pool(name="psv", bufs=1, space="PSUM"))
        psx = ctx.enter_context(tc.tile_pool(name="psx", bufs=1, space="PSUM"))

        # ---- persistent SBUF tensors ------------------------------------
        xs = [per.tile([128, SEQ], dt.bfloat16, name=f"x{c}", tag=f"x{c}")
              for c in range(3)]
        qTd = [per.tile([HD, 512], dt.bfloat16, name=f"q{n}", tag=f"q{n}")
               for n in range(NCHUNK)]
        kTd = [per.tile([HD, 512], dt.bfloat16, name=f"k{n}", tag=f"k{n}")
               for n in range(NCHUNK)]
        v_aug = per.tile([128, 64 * 49], dt.bfloat16, name="vaug", tag="vaug")
        v_aug3 = v_aug[:].rearrange("p (t c) -> p t c", c=49)
        vpad = per.tile([128, PADN], dt.bfloat16, name="vpad", tag="vpad")
        vp3 = vpad[:].rearrange("p (a b) -> p a b", b=PADW)

        wall = per.tile([128, WN], dt.bfloat16, name="wall", tag="wall")
        wq_sb = [wall[:, c * HD:(c + 1) * HD] for c in range(3)]
        wk_sb = [wall[:, 144 + c * HD:144 + (c + 1) * HD] for c in range(3)]
        wv_sb = [wall[:, 288 + c * 49:288 + (c + 1) * 49] for c in range(3)]
        dg_sb = wall[:, 435:867]
        A0 = 867
        bv_row = wall[0:1, A0 + 560:A0 + 609]
        lb_row = wall[0:1, A0 + 609:A0 + 657]
        ones128 = wall[0:1, A0 + 657:A0 + 785]
        bcol = per.tile([128, 2], dt.float32, name="bcol", tag="bcol")
        bq_col = bcol[0:HD, 0:1]
        bv_col = bcol[0:HD, 1:2]

        # ---- input DMAs (spread across queues; x first on SP) -----------
        # SP queue: x in 512-col minis for the first half (c-interleaved so
        # projection chunks complete early), then two big quarters
        for m in range(8):
            for c in range(3):
                nc.sync.dma_start(xs[c][:, m * 512:(m + 1) * 512],
                                  xT_d[c, :, m * 512:(m + 1) * 512])
        for j in range(2, 4):
            for c in range(3):
                nc.sync.dma_start(xs[c][:, j * 2048:(j + 1) * 2048],
                                  xT_d[c, :, j * 2048:(j + 1) * 2048])
        # ACT queue: the single packed weight wall + fp32 bias columns
        nc.scalar.dma_start(wall[:], wall_d[:])
        nc.scalar.dma_start(bcol[:], bcol_d[:])
        warm = per.tile([128, 2], dt.bfloat16, name="warm", tag="warm")
        nc.scalar.activation(warm[:], bcol[:], AF.Exp)  # preload Exp table

        # zero the padded image (borders must be 0; interior overwritten);
        # partition rows 48/112 are all-ones: the center LePE tap contracts
        # over 49 partitions so the ones row adds the lb bias (from dg row 48)
        nc.vector.memset(vpad[:], 0.0)
        nc.scalar.dma_start(vpad[48:49, :], vone_d[:])
        nc.scalar.dma_start(vpad[112:113, :], vone_d[:])

        # ---- projection units (sharing one persistent PSUM bank; q/k/vT
        # alternate partition halves so consecutive units don't WAR) -------
        px = psx.tile([128, 512], dt.float32, name="px", tag="px")
        half = [0]

        def nexthalf():
            half[0] ^= 64
            return half[0]

        def unit_q(n):
            """qT for chunk n: 3 proj matmuls; bias folded into the copy."""
            rb = nexthalf()
            reg = px[rb:rb + HD, :]
            sl = slice(n * 512, (n + 1) * 512)
            for c in range(3):
                nc.tensor.matmul(reg, wq_sb[c], xs[c][:, sl],
                                 start=(c == 0), stop=(c == 2),
                                 skip_group_check=True)
            nc.vector.tensor_scalar(qTd[n][:], reg, bq_col, None, op0=ALU.add)

        def unit_k(n):
            """kT for chunk n (no bias -- softmax-invariant)."""
            rb = nexthalf()
            reg = px[rb:rb + HD, :]
            sl = slice(n * 512, (n + 1) * 512)
            for c in range(3):
                nc.tensor.matmul(reg, wk_sb[c], xs[c][:, sl],
                                 start=(c == 0), stop=(c == 2),
                                 skip_group_check=True)
            nc.vector.tensor_copy(kTd[n][:], reg)

        def unit_vT(n):
            """channel-major v for chunk n (8 image rows) -> vpad scatter."""
            rb = nexthalf()
            reg = px[rb:rb + HD, :]
            sl = slice(n * 512, (n + 1) * 512)
            for c in range(3):
                nc.tensor.matmul(reg, wv_sb[c][:, 0:HD], xs[c][:, sl],
                                 start=(c == 0), stop=(c == 2),
                                 skip_group_check=True)
            ib = 64 * (n // 8)
            r0 = 8 * (n % 8)
            nc.vector.tensor_scalar(vp3[ib:ib + HD, 1 + r0:1 + r0 + 8, 1:65],
                                    reg, bv_col, None, op0=ALU.add)

        vacol = [0]

        def unit_vA(g):
            """v_aug tiles 4g..4g+3 (seq-major, ones column via bias mm)."""
            vacol[0] ^= 256
            c0 = vacol[0]
            for i in range(4):
                t = 4 * g + i
                reg = px[:, c0 + i * 49:c0 + i * 49 + 49]
                for c in range(3):
                    nc.tensor.matmul(reg, xs[c][:, t * 128:(t + 1) * 128],
                                     wv_sb[c], start=(c == 0), stop=False,
                                     skip_group_check=True)
                nc.tensor.matmul(reg, ones128, bv_row,
                                 start=False, stop=True, skip_group_check=True)
            nc.vector.tensor_copy(v_aug3[:, 4 * g:4 * g + 4, :],
                                  px[:, c0:c0 + 196])

        def emit_unit(kind, n):
            if kind == "q":
                unit_q(n)
            elif kind == "k":
                unit_k(n)
            elif kind == "vA":
                unit_vA(n)
            else:
                unit_vT(n)

        # unit queue consumed between score groups of the main loop, ordered
        # to match need()-order exactly so prefix-pops stay 1-2 units/slot.
        units = [("vA", 0)]
        for n in range(1, 8):
            units.append(("k", n))
            units.append(("vA", n))
            if n == 5:
                units.append(("q", 1))
        units += [("vT", 0), ("vT", 1)]
        for n in range(2, 8):
            units.append(("q", n))
            units.append(("vT", n))
        for n in range(8, 16):
            units.append(("k", n))
            units.append(("vA", n))
            units.append(("q", n))
        for n in range(8, 16):
            units.append(("vT", n))

        emitted = set()

        def need(kind, n):
            kn = (kind, n)
            if kn in emitted or kn not in units:
                return
            while units:
                u = units.pop(0)
                emit_unit(*u)
                emitted.add(u)
                if u == kn:
                    return

        def drain(k=1):
            for _ in range(k):
                if units:
                    u = units.pop(0)
                    emit_unit(*u)
                    emitted.add(u)

        # ---- pre-main prologue (just enough for the first score group) --
        unit_q(0); emitted.add(("q", 0))
        unit_k(0); emitted.add(("k", 0))

        # ---- main loop ---------------------------------------------------
        groups = [list(range(s, min(s + 3, KT_PER_B))) for s in range(0, KT_PER_B, 3)]
        NG = len(groups)
        taps = [(dr, dc) for dr in (-1, 0, 1) for dc in (-1, 0, 1)]

        def lepe_and_epilogue(cc, pv, last=False):
            """LePE diag matmuls into pv region B, then divide+add epilogue.
            Emitted early in chunk cc+1 so the next scores aren't delayed."""
            P0 = 64 * (cc // 8)
            for qb in range(4):
                r0 = 8 * (cc % 8) + 2 * qb
                reg = pv[:, 196 + qb * HD:196 + (qb + 1) * HD]
                # per-row taps: walrus requires 1-free-dim weight APs, so each
                # 128-pixel q-block takes its two image rows separately
                for ti, (dr, dc) in enumerate(taps):
                    # center tap contracts 49 partitions: ch + the ones row,
                    # whose dg row carries lb (folds the conv bias in)
                    w = HD + 1 if ti == 4 else HD
                    dgs = dg_sb[P0:P0 + w, ti * HD:(ti + 1) * HD]
                    for rr in range(2):
                        lhs = vp3[P0:P0 + w, 1 + r0 + rr + dr,
                                  1 + dc:1 + dc + 64]
                        # ti==0 starts BOTH row-halves: partition-blind PSUM
                        # zero-region bookkeeping requires the halves to move
                        # in lockstep (start replaces; later taps accumulate)
                        out_rr = pv[64 * rr:64 * rr + 64,
                                    196 + qb * HD:196 + (qb + 1) * HD]
                        nc.tensor.matmul(out_rr, lhs, dgs,
                                         start=(ti == 0), stop=(ti == 8 and rr == 1),
                                         skip_group_check=True)
            rec = rot.tile([128, 4], dt.float32, name="rec", tag="rec")
            ot = rot.tile([128, 192], dt.float32, name="ot", tag="ot")
            pv3 = pv[:, 0:196].rearrange("p (a b) -> p a b", b=49)
            nc.vector.reciprocal(rec[:], pv3[:, :, 48:49])
            for qb in range(4):
                nc.vector.tensor_scalar(ot[:, qb * HD:(qb + 1) * HD],
                                        pv[:, qb * 49:qb * 49 + HD],
                                        rec[:, qb:qb + 1], None, op0=ALU.mult)
            nc.vector.tensor_add(ot[:], ot[:], pv[:, 196:388])
            dma = nc.scalar.dma_start if last else nc.gpsimd.dma_start
            dma(out_v[:, 4 * cc:4 * cc + 4, :],
                ot[:].rearrange("p (t c) -> p t c", c=HD))

        pending = None
        for cc in range(NCHUNK):
            bc = cc // 8
            need("q", cc)   # safety; normally emitted mid-previous-chunk
            # full-bank tile: partition-offset PSUM writes require a 2048B
            # row pitch for correct zero-region accounting
            pv = psv.tile([128, 512], dt.float32, name="pv", tag="pv")
            pts = {}

            def pv_group(g):
                ptg = pts.pop(g)
                for j, kt in enumerate(groups[g]):
                    for qb in range(4):
                        # start=True only on the very first matmul: the PSUM
                        # zero-region is the whole 2KB bank per partition, so
                        # later starts would wipe sibling regions' accumulation
                        nc.tensor.matmul(pv[0:128, qb * 49:qb * 49 + 49],
                                         ptg[:, j * 512 + qb * 128:
                                             j * 512 + (qb + 1) * 128],
                                         v_aug3[:, bc * 32 + kt, :],
                                         start=(kt == 0 and qb == 0),
                                         stop=(kt == KT_PER_B - 1),
                                         skip_group_check=True)

            for gi, kts in enumerate(groups):
                st = pss.tile([128, 1536], dt.float32, name="st", tag="st")
                for j, kt in enumerate(kts):
                    ktile = kTd[bc * 8 + kt // 4]
                    koff = (kt % 4) * 128
                    nc.tensor.matmul(st[:, j * 512:(j + 1) * 512],
                                     ktile[:, koff:koff + 128],
                                     qTd[cc][:], skip_group_check=True)
                w = 512 * len(kts)
                pt = ptp.tile([128, 1536], dt.bfloat16, name="pt", tag="pt")
                nc.scalar.activation(pt[:, 0:w], st[:, 0:w], AF.Exp)
                pts[gi] = pt
                # previous chunk's LePE + epilogue, after this chunk's first
                # scores so ACT rolls straight into the next exp; PV lags one
                # group so unit stalls never sit ahead of the next scores
                if gi == 0:
                    if pending is not None:
                        lepe_and_epilogue(*pending)
                        pending = None
                else:
                    pv_group(gi - 1)
                # lookahead: vA for this group's (lagged) PV, k for the next
                # group's scores -- emitted after the scores they could stall
                need("vA", (bc * 32 + kts[-1]) // 4)
                if gi == 5 and cc + 1 < NCHUNK:
                    need("q", cc + 1)
                if gi + 1 < NG:
                    need("k", bc * 8 + groups[gi + 1][-1] // 4)
                elif cc + 1 < NCHUNK:
                    nb = (cc + 1) // 8
                    need("k", nb * 8)
                    if cc % 8 < 7:
                        need("vT", cc + 1)
                    elif cc == 7:
                        need("vT", 9)
                if cc > 0:
                    drain(1)
            pv_group(NG - 1)
            pending = (cc, pv)

        lepe_and_epilogue(pending[0], pending[1], last=True)

    nc.compile()
    return nc


def _prep_in_maps(x, qkv_w, qkv_b, lepe_w, lepe_b):
    bf16 = ml_dtypes.bfloat16
    X = np.asarray(x, dtype=np.float32).reshape(SEQ, DIM)
    xT = np.ascontiguousarray(X.T).astype(bf16).reshape(3, 128, SEQ)

    qkv_w = np.asarray(qkv_w, dtype=np.float32)
    qkv_b = np.asarray(qkv_b, dtype=np.float32)
    lepe_w = np.asarray(lepe_w, dtype=np.float32)
    lepe_b = np.asarray(lepe_b, dtype=np.float32)

    WN = 144 + 144 + 147 + 432 + 785
    in_maps = []
    for h in range(NUM_HEADS):
        sl = slice(h * HD, (h + 1) * HD)
        wq = qkv_w[sl, :] * SCALE                    # [48, 384]
        wk = qkv_w[DIM + h * HD:DIM + (h + 1) * HD, :]
        wv = qkv_w[2 * DIM + h * HD:2 * DIM + (h + 1) * HD, :]
        bq = qkv_b[sl] * SCALE
        bv = qkv_b[2 * DIM + h * HD:2 * DIM + (h + 1) * HD]
        lb = lepe_b[sl]
        lw = lepe_w[sl, 0].reshape(HD, 3, 3)     # [48, dr, dc]

        wall = np.zeros((128, WN), dtype=np.float32)
        for c in range(3):
            wall[:, c * HD:(c + 1) * HD] = wq.T[c * 128:(c + 1) * 128]
            wall[:, 144 + c * HD:144 + (c + 1) * HD] = wk.T[c * 128:(c + 1) * 128]
            wall[:, 288 + c * 49:288 + c * 49 + HD] = wv.T[c * 128:(c + 1) * 128]
        for ti in range(9):
            dr, dc = ti // 3, ti % 3
            d = np.diag(lw[:, dr, dc])
            wall[0:HD, 435 + ti * HD:435 + (ti + 1) * HD] = d
            wall[64:64 + HD, 435 + ti * HD:435 + (ti + 1) * HD] = d
        wall[HD, 435 + 4 * HD:435 + 5 * HD] = lb
        wall[64 + HD, 435 + 4 * HD:435 + 5 * HD] = lb
        A0 = 867
        wall[0, A0:A0 + 512] = 1.0
        wall[0, A0 + 512:A0 + 560] = bq
        wall[0, A0 + 560:A0 + 608] = bv
        wall[0, A0 + 608] = 1.0                  # ones column of v_aug
        wall[0, A0 + 609:A0 + 657] = lb
        wall[0, A0 + 657:A0 + 785] = 1.0

        bcol = np.zeros((128, 2), dtype=np.float32)
        bcol[0:HD, 0] = bq
        bcol[0:HD, 1] = bv
        vone = np.ones((1, PADN), dtype=np.float32)
        in_maps.append({"xT": xT, "wall": wall.astype(bf16), "bcol": bcol,
                        "vone": vone.astype(bf16)})
    return in_maps


def kernel(x, qkv_w, qkv_b, lepe_w, lepe_b, H=64, W=64):
    assert int(H) == 64 and int(W) == 64
    from concourse.bass_utils import run_bass_kernel_spmd

    if "nc" not in _CACHE:
        _CACHE["nc"] = _build_module()
    nc = _CACHE["nc"]

    in_maps = _prep_in_maps(x, qkv_w, qkv_b, lepe_w, lepe_b)
    res = run_bass_kernel_spmd(nc, in_maps, core_ids=list(range(NUM_HEADS)))

    full = np.empty((SEQ, DIM), dtype=np.float32)
    for h in range(NUM_HEADS):
        full[:, h * HD:(h + 1) * HD] = res.results[h]["out"].reshape(SEQ, HD)
    return full.reshape(B, N, DIM)


# revision 35
# speedup vs baseline: 1.4769x; 1.0183x over previous
"""Trainium2 Bass kernel for nn_Attention_13829794693777.

Multi-head attention (8 heads, head_dim 48) + LePE depthwise 3x3 conv on v.
Sharding: tensor-parallel over heads -- one head per NeuronCore (8 cores).
Each core gets the full (transposed, bf16) input activations plus its head's
qkv weight slice and LePE channel slice; it produces the [seq, 48] channel
slice of the output. The host concatenates slices and reshapes.

Device-side dataflow (per core / head), designed to keep the Activation
engine (the exp() throughput floor, ~250us modeled) busy from ~6us onward:

  1. Projections are W-stationary matmuls into a single shared PSUM bank
     (psX), with biases folded into extra rank-1 matmuls (ones-vector
     tricks). The k bias is dropped entirely (softmax-invariant).
     q/k land channel-major [48, 512] per 512-pixel chunk -> SBUF bf16.
     v lands twice: seq-major v_aug tiles [128 pix, 49] (last col = 1.0
     via the bias matmul) for PV, and channel-major [48, 512] scattered
     into a zero-padded 66x66 image (vpad, bf16) for LePE.
  2. Scores transposed: st[k,q] = k-block(48x128).T @ qT(48x512), fp32
     PSUM, 3 k-tiles per group (two rotating 3-bank st tiles). exp on ACT
     (no max subtraction; |S| <= ~15) -> P^T bf16 in SBUF.
  3. PV P-stationary: out[q,ch] += ptBlock(128k x 128q).T @ v_aug(128k,49).
     Cost is 49 columns per matmul; the ones column yields softmax
     denominators at col 48. All 32 k-tiles accumulate into one PSUM bank
     (4 q-blocks x 49), already seq-major -- no output transposes.
  4. LePE: 9 diagonal-weight matmuls per q-block (diag(w_tap) as moving
     rhs, shifted padded-image view as stationary lhsT) + bias matmul,
     accumulated into region B of the same PSUM bank. Zero DVE tap work.
  5. Epilogue per 512-query chunk: reciprocal of denominators (DVE), then
     4x AFFINE_THEN_ADD: out = attn*rec + lepe -> SBUF -> DMA out.

Emission order interleaves batch-1 projections (and late batch-0 units)
between chunk score groups so the PE never starves ACT.
"""

import numpy as np
import ml_dtypes

NUM_HEADS = 8
DIM = 384
HD = 48
B = 2
N = 4096
SEQ = B * N          # 8192
IMG = 64             # H = W = 64
PADW = IMG + 2       # 66
PADN = PADW * PADW   # 4356
SCALE = HD ** -0.5
NCHUNK = SEQ // 512  # 16 query chunks of 512
KT_PER_B = N // 128  # 32 k-tiles per batch

_CACHE = {}


def _build_module():
    """Build (once) the Bacc module shared by all 8 cores."""
    import concourse.bacc as bacc
    import concourse.mybir as mybir
    import concourse.tile as tile
    from concourse.dve_ops import AFFINE_THEN_ADD
    from contextlib import ExitStack

    dt = mybir.dt
    AF = mybir.ActivationFunctionType
    ALU = mybir.AluOpType

    nc = bacc.Bacc("TRN2", target_bir_lowering=False, debug=False, num_devices=8)

    # ---- DRAM parameters -------------------------------------------------
    xT_d = nc.dram_tensor("xT", [3, 128, SEQ], dt.bfloat16, kind="ExternalInput").ap()
    # wall packs every weight: wq(3x48)|wk(3x48)|wv(3x49)|dg(9x48)|aux(785,row0)
    # + two per-partition bias columns (bq, bv) folded into the PSUM copies
    WN = 144 + 144 + 147 + 432 + 785
    wall_d = nc.dram_tensor("wall", [128, WN], dt.bfloat16, kind="ExternalInput").ap()
    bcol_d = nc.dram_tensor("bcol", [128, 2], dt.float32, kind="ExternalInput").ap()
    vone_d = nc.dram_tensor("vone", [1, PADN], dt.bfloat16, kind="ExternalInput").ap()
    out_d = nc.dram_tensor("out", [64, 128, HD], dt.float32, kind="ExternalOutput").ap()
    out_v = out_d.rearrange("t p c -> p t c")

    with tile.TileContext(nc) as tc, ExitStack() as ctx:
        per = ctx.enter_context(tc.tile_pool(name="per", bufs=1))
        rot = ctx.enter_context(tc.tile_pool(name="rot", bufs=2))
        ptp = ctx.enter_context(tc.tile_pool(name="ptp", bufs=3))
        pss = ctx.enter_context(tc.tile_pool(name="pss", bufs=2, space="PSUM"))
        psv = ctx.enter_context(tc.tile_pool(name="psv", bufs=1, space="PSUM"))
        psx = ctx.enter_context(tc.tile_pool(name="psx", bufs=1, space="PSUM"))

        # ---- persistent SBUF tensors ------------------------------------
        xs = [per.tile([128, SEQ], dt.bfloat16, name=f"x{c}", tag=f"x{c}")
              for c in range(3)]
        qTd = [per.tile([HD, 512], dt.bfloat16, name=f"q{n}", tag=f"q{n}")
               for n in range(NCHUNK)]
        kTd = [per.tile([HD, 512], dt.bfloat16, name=f"k{n}", tag=f"k{n}")
               for n in range(NCHUNK)]
        v_aug = per.tile([128, 64 * 49], dt.bfloat16, name="vaug", tag="vaug")
        v_aug3 = v_aug[:].rearrange("p (t c) -> p t c", c=49)
        vpad = per.tile([128, PADN], dt.bfloat16, name="vpad", tag="vpad")
        vp3 = vpad[:].rearrange("p (a b) -> p a b", b=PADW)

        wall = per.tile([128, WN], dt.bfloat16, name="wall", tag="wall")
        wq_sb = [wall[:, c * HD:(c + 1) * HD] for c in range(3)]
        wk_sb = [wall[:, 144 + c * HD:144 + (c + 1) * HD] for c in range(3)]
        wv_sb = [wall[:, 288 + c * 49:288 + (c + 1) * 49] for c in range(3)]
        dg_sb = wall[:, 435:867]
        A0 = 867
        bv_row = wall[0:1, A0 + 560:A0 + 609]
        lb_row = wall[0:1, A0 + 609:A0 + 657]
        ones128 = wall[0:1, A0 + 657:A0 + 785]
        bcol = per.tile([128, 2], dt.float32, name="bcol", tag="bcol")
        bq_col = bcol[0:HD, 0:1]
        bv_col = bcol[0:HD, 1:2]

        # zero the padded image first (borders must be 0); partition rows
        # 48/112 are overwritten to ones by the vone DMAs below: the center
        # LePE tap contracts 49 partitions so the ones row adds lb (dg row 48)
        nc.vector.memset(vpad[:], 0.0)

        # ---- input DMAs (spread across queues; x first on SP) -----------
        # SP queue: x in 512-col minis for the first half (c-interleaved so
        # projection chunks complete early), then two big quarters
        for m in range(8):
            for c in range(3):
                nc.sync.dma_start(xs[c][:, m * 512:(m + 1) * 512],
                                  xT_d[c, :, m * 512:(m + 1) * 512])
            if m == 0:
                # LePE ones rows (read first at chunk 1's flush, ~30us in)
                nc.sync.dma_start(vpad[48:49, :], vone_d[:])
                nc.sync.dma_start(vpad[112:113, :], vone_d[:])
        for j in range(2, 4):
            for c in range(3):
                nc.sync.dma_start(xs[c][:, j * 2048:(j + 1) * 2048],
                                  xT_d[c, :, j * 2048:(j + 1) * 2048])
        # ACT queue: the single packed weight wall + fp32 bias columns
        nc.scalar.dma_start(wall[:], wall_d[:])
        nc.scalar.dma_start(bcol[:], bcol_d[:])
        warm = per.tile([128, 2], dt.bfloat16, name="warm", tag="warm")
        nc.scalar.activation(warm[:], bcol[:], AF.Exp)  # preload Exp table

        # ---- projection units (sharing one persistent PSUM bank; q/k/vT
        # alternate partition halves so consecutive units don't WAR) -------
        px = psx.tile([128, 512], dt.float32, name="px", tag="px")
        half = [0]

        def nexthalf():
            half[0] ^= 64
            return half[0]

        def unit_q(n):
            """qT for chunk n: 3 proj matmuls; bias folded into the copy."""
            rb = nexthalf()
            reg = px[rb:rb + HD, :]
            sl = slice(n * 512, (n + 1) * 512)
            for c in range(3):
                nc.tensor.matmul(reg, wq_sb[c], xs[c][:, sl],
                                 start=(c == 0), stop=(c == 2),
                                 skip_group_check=True)
            nc.vector.tensor_scalar(qTd[n][:], reg, bq_col, None, op0=ALU.add)

        def unit_k(n):
            """kT for chunk n (no bias -- softmax-invariant)."""
            rb = nexthalf()
            reg = px[rb:rb + HD, :]
            sl = slice(n * 512, (n + 1) * 512)
            for c in range(3):
                nc.tensor.matmul(reg, wk_sb[c], xs[c][:, sl],
                                 start=(c == 0), stop=(c == 2),
                                 skip_group_check=True)
            nc.vector.tensor_copy(kTd[n][:], reg)

        def unit_vT(n):
            """channel-major v for chunk n (8 image rows) -> vpad scatter."""
            rb = nexthalf()
            reg = px[rb:rb + HD, :]
            sl = slice(n * 512, (n + 1) * 512)
            for c in range(3):
                nc.tensor.matmul(reg, wv_sb[c][:, 0:HD], xs[c][:, sl],
                                 start=(c == 0), stop=(c == 2),
                                 skip_group_check=True)
            ib = 64 * (n // 8)
            r0 = 8 * (n % 8)
            nc.vector.tensor_scalar(vp3[ib:ib + HD, 1 + r0:1 + r0 + 8, 1:65],
                                    reg, bv_col, None, op0=ALU.add)

        vacol = [0]

        def unit_vA(g):
            """v_aug tiles 4g..4g+3 (seq-major, ones column via bias mm)."""
            vacol[0] ^= 256
            c0 = vacol[0]
            for i in range(4):
                t = 4 * g + i
                reg = px[:, c0 + i * 49:c0 + i * 49 + 49]
                for c in range(3):
                    nc.tensor.matmul(reg, xs[c][:, t * 128:(t + 1) * 128],
                                     wv_sb[c], start=(c == 0), stop=False,
                                     skip_group_check=True)
                nc.tensor.matmul(reg, ones128, bv_row,
                                 start=False, stop=True, skip_group_check=True)
            nc.vector.tensor_copy(v_aug3[:, 4 * g:4 * g + 4, :],
                                  px[:, c0:c0 + 196])

        def emit_unit(kind, n):
            if kind == "q":
                unit_q(n)
            elif kind == "k":
                unit_k(n)
            elif kind == "vA":
                unit_vA(n)
            else:
                unit_vT(n)

        # unit queue consumed between score groups of the main loop, ordered
        # to match need()-order exactly so prefix-pops stay 1-2 units/slot.
        units = [("vA", 0)]
        for n in range(1, 8):
            units.append(("k", n))
            units.append(("vA", n))
            if n == 5:
                units.append(("q", 1))
        units += [("vT", 0), ("vT", 1)]
        for n in range(2, 8):
            units.append(("q", n))
            units.append(("vT", n))
        for n in range(8, 16):
            units.append(("k", n))
            units.append(("vA", n))
            units.append(("q", n))
        for n in range(8, 16):
            units.append(("vT", n))

        emitted = set()

        def need(kind, n):
            kn = (kind, n)
            if kn in emitted or kn not in units:
                return
            while units:
                u = units.pop(0)
                emit_unit(*u)
                emitted.add(u)
                if u == kn:
                    return

        def drain(k=1):
            for _ in range(k):
                if units:
                    u = units.pop(0)
                    emit_unit(*u)
                    emitted.add(u)

        # ---- pre-main prologue (just enough for the first score group) --
        unit_q(0); emitted.add(("q", 0))
        unit_k(0); emitted.add(("k", 0))

        # ---- main loop ---------------------------------------------------
        groups = [list(range(s, min(s + 3, KT_PER_B))) for s in range(0, KT_PER_B, 3)]
        NG = len(groups)
        taps = [(dr, dc) for dr in (-1, 0, 1) for dc in (-1, 0, 1)]

        # chunks whose LePE is emitted mid-chunk (one q-block per slot at
        # g5..g8); earlier chunks defer it to the next chunk's g0 because
        # their vT scatters aren't emitted yet (unit queue still draining)
        MIDLEPE = 6

        def lepe_qb(cc, pv, qb):
            """LePE taps for one q-block into pv region B (pre-zeroed by the
            slot-g1 memset; all taps start=False so the partition-blind PSUM
            zero-region bookkeeping stays consistent)."""
            P0 = 64 * (cc // 8)
            r0 = 8 * (cc % 8) + 2 * qb
            for ti, (dr, dc) in enumerate(taps):
                # center tap contracts 49 partitions: ch + the ones row,
                # whose dg row carries lb (folds the conv bias in)
                w = HD + 1 if ti == 4 else HD
                dgs = dg_sb[P0:P0 + w, ti * HD:(ti + 1) * HD]
                for rr in range(2):
                    lhs = vp3[P0:P0 + w, 1 + r0 + rr + dr, 1 + dc:1 + dc + 64]
                    out_rr = pv[64 * rr:64 * rr + 64,
                                196 + qb * HD:196 + (qb + 1) * HD]
                    nc.tensor.matmul(out_rr, lhs, dgs,
                                     start=False, stop=False,
                                     skip_group_check=True)

        def epilogue(cc, pv, last=False):
            rec = rot.tile([128, 4], dt.float32, name="rec", tag="rec")
            ot = rot.tile([128, 192], dt.float32, name="ot", tag="ot")
            pv3 = pv[:, 0:196].rearrange("p (a b) -> p a b", b=49)
            nc.vector.reciprocal(rec[:], pv3[:, :, 48:49])
            for qb in range(4):
                nc.vector.tensor_scalar(ot[:, qb * HD:(qb + 1) * HD],
                                        pv[:, qb * 49:qb * 49 + HD],
                                        rec[:, qb:qb + 1], None, op0=ALU.mult)
            nc.vector.tensor_add(ot[:], ot[:], pv[:, 196:388])
            dma = nc.scalar.dma_start if last else nc.gpsimd.dma_start
            dma(out_v[:, 4 * cc:4 * cc + 4, :],
                ot[:].rearrange("p (t c) -> p t c", c=HD))

        def mk_pv_group(pv, bc):
            def pv_group(g, ptg):
                for j, kt in enumerate(groups[g]):
                    for qb in range(4):
                        # start=True only on the very first matmul: the PSUM
                        # zero-region is the whole 2KB bank per partition, so
                        # later starts would wipe sibling regions' accumulation
                        nc.tensor.matmul(pv[0:128, qb * 49:qb * 49 + 49],
                                         ptg[:, j * 512 + qb * 128:
                                             j * 512 + (qb + 1) * 128],
                                         v_aug3[:, bc * 32 + kt, :],
                                         start=(kt == 0 and qb == 0),
                                         stop=(kt == KT_PER_B - 1),
                                         skip_group_check=True)
            return pv_group

        pending = None   # (cc, pv, pv_group, pt_g10): flushed at next g0
        for cc in range(NCHUNK):
            bc = cc // 8
            need("q", cc)   # safety; normally emitted mid-previous-chunk
            # full-bank tile: partition-offset PSUM writes require a 2048B
            # row pitch for correct zero-region accounting
            pv = psv.tile([128, 512], dt.float32, name="pv", tag="pv")
            pv_group = mk_pv_group(pv, bc)
            pts = {}

            for gi, kts in enumerate(groups):
                st = pss.tile([128, 1536], dt.float32, name="st", tag="st")
                for j, kt in enumerate(kts):
                    ktile = kTd[bc * 8 + kt // 4]
                    koff = (kt % 4) * 128
                    nc.tensor.matmul(st[:, j * 512:(j + 1) * 512],
                                     ktile[:, koff:koff + 128],
                                     qTd[cc][:], skip_group_check=True)
                w = 512 * len(kts)
                pt = ptp.tile([128, 1536], dt.bfloat16, name="pt", tag="pt")
                nc.scalar.activation(pt[:, 0:w], st[:, 0:w], AF.Exp)
                pts[gi] = pt
                # previous chunk's held PV tail + (early chunks) LePE +
                # epilogue, after this chunk's first scores so ACT rolls
                # straight into the next exp; PV lags one group so unit
                # stalls never sit ahead of the next scores
                if gi == 0:
                    if pending is not None:
                        pcc, ppv, ppvg, ppt = pending
                        ppvg(NG - 1, ppt)
                        if pcc < MIDLEPE:
                            for qb in range(4):
                                lepe_qb(pcc, ppv, qb)
                        epilogue(pcc, ppv)
                        pending = None
                else:
                    pv_group(gi - 1, pts.pop(gi - 1))
                if gi == 1:
                    # zero region B so LePE taps can accumulate start-free
                    nc.vector.memset(pv[:, 196:388], 0.0)
                if cc >= MIDLEPE and 5 <= gi <= 8:
                    lepe_qb(cc, pv, gi - 5)
                # lookahead: vA for this group's (lagged) PV, k for the next
                # group's scores -- emitted after the scores they could stall
                need("vA", (bc * 32 + kts[-1]) // 4)
                if gi == 5 and cc + 1 < NCHUNK:
                    need("q", cc + 1)
                if gi + 1 < NG:
                    need("k", bc * 8 + groups[gi + 1][-1] // 4)
                elif cc + 1 < NCHUNK:
                    nb = (cc + 1) // 8
                    need("k", nb * 8)
                    if cc % 8 < 7:
                        need("vT", cc + 1)
                    elif cc == 7:
                        need("vT", 9)
                if cc > 0 and gi < NG - 1:
                    drain(2 if gi == 1 else 1)
            pending = (cc, pv, pv_group, pts.pop(NG - 1))

        pcc, ppv, ppvg, ppt = pending
        ppvg(NG - 1, ppt)
        epilogue(pcc, ppv, last=True)

    nc.compile()
    return nc


def _prep_in_maps(x, qkv_w, qkv_b, lepe_w, lepe_b):
    bf16 = ml_dtypes.bfloat16
    X = np.asarray(x, dtype=np.float32).reshape(SEQ, DIM)
    xT = np.ascontiguousarray(X.T).astype(bf16).reshape(3, 128, SEQ)

    qkv_w = np.asarray(qkv_w, dtype=np.float32)
    qkv_b = np.asarray(qkv_b, dtype=np.float32)
    lepe_w = np.asarray(lepe_w, dtype=np.float32)
    lepe_b = np.asarray(lepe_b, dtype=np.float32)

    WN = 144 + 144 + 147 + 432 + 785
    in_maps = []
    for h in range(NUM_HEADS):
        sl = slice(h * HD, (h + 1) * HD)
        wq = qkv_w[sl, :] * SCALE                    # [48, 384]
        wk = qkv_w[DIM + h * HD:DIM + (h + 1) * HD, :]
        wv = qkv_w[2 * DIM + h * HD:2 * DIM + (h + 1) * HD, :]
        bq = qkv_b[sl] * SCALE
        bv = qkv_b[2 * DIM + h * HD:2 * DIM + (h + 1) * HD]
        lb = lepe_b[sl]
        lw = lepe_w[sl, 0].reshape(HD, 3, 3)     # [48, dr, dc]

        wall = np.zeros((128, WN), dtype=np.float32)
        for c in range(3):
            wall[:, c * HD:(c + 1) * HD] = wq.T[c * 128:(c + 1) * 128]
            wall[:, 144 + c * HD:144 + (c + 1) * HD] = wk.T[c * 128:(c + 1) * 128]
            wall[:, 288 + c * 49:288 + c * 49 + HD] = wv.T[c * 128:(c + 1) * 128]
        for ti in range(9):
            dr, dc = ti // 3, ti % 3
            d = np.diag(lw[:, dr, dc])
            wall[0:HD, 435 + ti * HD:435 + (ti + 1) * HD] = d
            wall[64:64 + HD, 435 + ti * HD:435 + (ti + 1) * HD] = d
        wall[HD, 435 + 4 * HD:435 + 5 * HD] = lb
        wall[64 + HD, 435 + 4 * HD:435 + 5 * HD] = lb
        A0 = 867
        wall[0, A0:A0 + 512] = 1.0
        wall[0, A0 + 512:A0 + 560] = bq
        wall[0, A0 + 560:A0 + 608] = bv
        wall[0, A0 + 608] = 1.0                  # ones column of v_aug
        wall[0, A0 + 609:A0 + 657] = lb
        wall[0, A0 + 657:A0 + 785] = 1.0

        bcol = np.zeros((128, 2), dtype=np.float32)
        bcol[0:HD, 0] = bq
        bcol[0:HD, 1] = bv
        vone = np.ones((1, PADN), dtype=np.float32)
        in_maps.append({"xT": xT, "wall": wall.astype(bf16), "bcol": bcol,
                        "vone": vone.astype(bf16)})
    return in_maps


def kernel(x, qkv_w, qkv_b, lepe_w, lepe_b, H=64, W=64):
    assert int(H) == 64 and int(W) == 64
    from concourse.bass_utils import run_bass_kernel_spmd

    if "nc" not in _CACHE:
        _CACHE["nc"] = _build_module()
    nc = _CACHE["nc"]

    in_maps = _prep_in_maps(x, qkv_w, qkv_b, lepe_w, lepe_b)
    res = run_bass_kernel_spmd(nc, in_maps, core_ids=list(range(NUM_HEADS)))

    full = np.empty((SEQ, DIM), dtype=np.float32)
    for h in range(NUM_HEADS):
        full[:, h * HD:(h + 1) * HD] = res.results[h]["out"].reshape(SEQ, HD)
    return full.reshape(B, N, DIM)
